# revision 19
# baseline (speedup 1.0000x reference)
"""Trainium2 Bass kernel for nn_Block_36438502540029 (involution CNN block).

Structure per core (data-parallel over batch, 2 images/core):
  conv1 (1x1, 512->128) -> ReLU -> padded bf16 buffer (quarter-major x
    layout so conv1 q0 starts after 1/4 of the x DMA)
  reduce (1x1, 128->32, replicated x4 on partitions) -> ReLU -> w1rep
  involution: 49 taps in 17 groups of <=3 (row-triplets + col-triplets).
    Per group/quarter the <=3 span matmuls (K=32, dup'd Ws rows) go to
    DISTINCT PE row-tiles via tile_position -> they execute concurrently
    (~3x span throughput). Maps land in a 3-bank PSUM tile (512-col
    strips). Three consume paths per group, chosen to balance engines:
      A: ACT drains strips to bf16 wm, DVE does merged 3-tap product
      G: ACT drains, gpsimd does the products (offload DVE)
      X: DVE multiplies the PSUM maps directly (fp32, no drain)
    Products accumulate into the out2 PSUM via PE identity matmuls
    (interleaved FIFO so PE stays busy between span groups).
  conv3 (1x1, 128->512) + identity residual via extra matmul + bias

All matmuls bf16 (full rate); PSUM accumulation fp32.
"""
import numpy as np
import ml_dtypes
from contextlib import ExitStack

import bass_rust
import concourse.bass as bass
import concourse.tile as tile
from concourse import bacc, mybir
from concourse import bass_utils

bf16 = mybir.dt.bfloat16
f32 = mybir.dt.float32
AF = mybir.ActivationFunctionType
ALU = mybir.AluOpType
BF = ml_dtypes.bfloat16

N_CORES = 8
B, CIN, H, W = 16, 512, 28, 28
BL = B // N_CORES            # images per core
CMID, CRED, G, GCH = 128, 32, 8, 16
KS, PD = 7, 3                # kernel size, pad
HWPX = H * W                 # 784
NPX = BL * HWPX              # 1568
PW = W + 2 * PD              # 34
PIMG = PW * PW               # 1156
NPAD = BL * PIMG + 8         # slack for strided views
NTAP = KS * KS               # 49
QW = HWPX // 2               # 392 px per quarter

# tap groups: row-triplets (taps t,t+1,t+2 in one kernel row; shift step 1)
# then col-triplets over the last kernel column (step 7 taps = PW shift)
GROUPS = []
for r in range(KS):
    GROUPS.append([7 * r + 0, 7 * r + 1, 7 * r + 2])
    GROUPS.append([7 * r + 3, 7 * r + 4, 7 * r + 5])
GROUPS.append([6, 13, 20])
GROUPS.append([27, 34, 41])
GROUPS.append([48])
NGRP = len(GROUPS)           # 17

# consume-path per group: X = direct-from-PSUM DVE, G = drain + gpsimd mul,
# everything else = drain + DVE mul
DIRECT_GROUPS = frozenset({1, 4, 7, 10, 13, 15})
GP_GROUPS = frozenset({2, 5, 8, 11, 14})
ACCUM_PACE = 3               # accum matmuls interleaved per span slot
FILLERS = True               # dependency-free PE matmuls against HAM throttle

_prog_cache = {}


def _sv(ap_base, pstride, offset, dims, ):
    """Strided view with explicit (step, num) free dims (may overlap)."""
    v = ap_base[:, offset:offset + 1].copy()
    v.ap = bass_rust.VecI64Pair([[pstride, 128]] + [list(d) for d in dims])
    return v


def _build_program(direct_groups=DIRECT_GROUPS, gp_groups=GP_GROUPS,
                   use_stt=False, fuse_y=True):
    nc = bacc.Bacc("TRN2", num_devices=N_CORES, debug=False)

    dr = {}
    dr["x"] = nc.dram_tensor("x", [128, 4 * NPX], bf16, kind="ExternalInput")
    dr["w1t"] = nc.dram_tensor("w1t", [128, 512], bf16, kind="ExternalInput")
    dr["wrt"] = nc.dram_tensor("wrt", [128, 128], bf16, kind="ExternalInput")
    dr["wsd"] = nc.dram_tensor("wsd", [96, NGRP * 128], bf16, kind="ExternalInput")
    dr["w3t"] = nc.dram_tensor("w3t", [128, 512], bf16, kind="ExternalInput")
    dr["ident"] = nc.dram_tensor("ident", [128, 128], bf16, kind="ExternalInput")
    dr["b1"] = nc.dram_tensor("b1", [128, 1], f32, kind="ExternalInput")
    dr["brr"] = nc.dram_tensor("brr", [128, 1], f32, kind="ExternalInput")
    dr["bsd"] = nc.dram_tensor("bsd", [128, NTAP], f32, kind="ExternalInput")
    dr["b3"] = nc.dram_tensor("b3", [128, 4], f32, kind="ExternalInput")
    y = nc.dram_tensor("y", [128, 4 * NPX], bf16, kind="ExternalOutput")

    with tile.TileContext(nc) as tc:
        with ExitStack() as ctx:
            const = ctx.enter_context(tc.tile_pool(name="const", bufs=1))
            sbuf = ctx.enter_context(tc.tile_pool(name="sbuf", bufs=1))
            wsmp = ctx.enter_context(tc.tile_pool(name="wsm", bufs=4))
            prodp = ctx.enter_context(tc.tile_pool(name="prod", bufs=8))
            ystg = ctx.enter_context(tc.tile_pool(name="ystg", bufs=2))
            pso = ctx.enter_context(tc.tile_pool(name="pso", bufs=1, space="PSUM"))
            psB = ctx.enter_context(tc.tile_pool(name="psB", bufs=2, space="PSUM"))

            # ---- loads: x quarter 0 first so conv1 starts ASAP ----
            # x layout: [(img,half) quarter q: 4][chunk k: 4][392], quarter-major
            xsb = sbuf.tile([128, 4 * NPX], bf16, name="xsb")
            nc.sync.dma_start(xsb[:, 0:NPX], dr["x"].ap()[:, 0:NPX])
            w1t_sb = const.tile([128, 512], bf16, name="w1t_sb")
            nc.scalar.dma_start(w1t_sb[:], dr["w1t"].ap())
            wrt_sb = const.tile([128, 128], bf16, name="wrt_sb")
            nc.gpsimd.dma_start(wrt_sb[:], dr["wrt"].ap())
            nc.scalar.dma_start(xsb[:, NPX:2 * NPX], dr["x"].ap()[:, NPX:2 * NPX])
            nc.gpsimd.dma_start(xsb[:, 2 * NPX:3 * NPX],
                                dr["x"].ap()[:, 2 * NPX:3 * NPX])
            wsd_sb = const.tile([128, NGRP * 128], bf16, name="wsd_sb")
            nc.sync.dma_start(wsd_sb[0:96, :], dr["wsd"].ap())
            nc.sync.dma_start(xsb[:, 3 * NPX:4 * NPX],
                              dr["x"].ap()[:, 3 * NPX:4 * NPX])
            id_sb = const.tile([128, 128], bf16, name="id_sb")
            nc.sync.dma_start(id_sb[:], dr["ident"].ap())
            w3t_sb = const.tile([128, 512], bf16, name="w3t_sb")
            nc.gpsimd.dma_start(w3t_sb[:], dr["w3t"].ap())
            b1_sb = const.tile([128, 1], f32, name="b1_sb")
            nc.sync.dma_start(b1_sb[:], dr["b1"].ap())
            brr_sb = const.tile([128, 1], f32, name="brr_sb")
            nc.sync.dma_start(brr_sb[:], dr["brr"].ap())
            bsd_sb = const.tile([128, NTAP], f32, name="bsd_sb")
            nc.sync.dma_start(bsd_sb[:], dr["bsd"].ap())
            b3_sb = const.tile([128, 4], f32, name="b3_sb")
            nc.sync.dma_start(b3_sb[:], dr["b3"].ap())

            pad_t = sbuf.tile([128, NPAD], bf16, name="pad_t")
            nc.vector.memset(pad_t[:], 0.0)
            pad4 = pad_t[:, 0:BL * PIMG].rearrange(
                "p (b i j) -> p b i j", b=BL, i=PW, j=PW)
            pad_ps = pad_t.ap[0][0]

            # ---- conv1 + reduce interleaved per quarter so the first
            # involution spans unblock after quarters 0-1 ----
            w1rep = sbuf.tile([128, NPX], bf16, name="w1rep")
            for q in range(4):
                b_, hh = q // 2, q % 2
                cps = psB.tile([128, 1536], f32, tag="bc", name=f"c1ps{q}")
                for k in range(4):
                    nc.tensor.matmul(
                        cps[:, 0:QW],
                        w1t_sb[:, 128 * k:128 * (k + 1)],
                        xsb[:, (4 * q + k) * QW:(4 * q + k + 1) * QW],
                        start=(k == 0), stop=(k == 3),
                    )
                rhs = pad4[:, b_:b_ + 1, PD + 14 * hh:PD + 14 * hh + 14, PD:PD + W]
                nc.scalar.activation(
                    rhs, cps[:, 0:QW].rearrange("p (a i j) -> p a i j",
                                                a=1, i=14, j=W),
                    AF.Relu, bias=b1_sb[:], scale=1.0,
                )
                rps = psB.tile([128, 1536], f32, tag="bc", name=f"redps{q}")
                nc.tensor.matmul(rps[:, 0:QW], wrt_sb, rhs,
                                 start=True, stop=True)
                nc.scalar.activation(
                    w1rep[:, QW * q:QW * (q + 1)], rps[:, 0:QW],
                    AF.Relu, bias=brr_sb[:], scale=1.0,
                )

            out2sb = sbuf.tile([128, NPX], bf16, name="out2sb")
            xsb_ps = xsb.ap[0][0]

            def conv3_img(img):
                # y_m[img] = W3'_m @ out2 + x_m + b3_m; quarter strips at 0/512
                for m in range(4):
                    c3 = psB.tile([128, 1536], f32, tag="bc", name=f"c3_{m}_{img}")
                    for s2 in range(2):
                        q = 2 * img + s2
                        nc.tensor.matmul(
                            c3[:, 512 * s2:512 * s2 + QW],
                            w3t_sb[:, 128 * m:128 * (m + 1)],
                            out2sb[:, HWPX * img + QW * s2:
                                   HWPX * img + QW * (s2 + 1)],
                            start=True, stop=False, skip_group_check=True,
                        )
                        nc.tensor.matmul(
                            c3[:, 512 * s2:512 * s2 + QW], id_sb,
                            xsb[:, (4 * q + m) * QW:(4 * q + m + 1) * QW],
                            start=False, stop=True, skip_group_check=True,
                        )
                    c3v = _sv(c3, c3.ap[0][0], 0, [(512, 2), (1, QW)])
                    ysb = ystg.tile([128, HWPX], bf16, tag="y",
                                    name=f"y{m}_{img}")
                    ysbv = ysb[:].rearrange("p (s n) -> p s n", s=2, n=QW)
                    if fuse_y and m % 2 == 1:
                        # b3 == 0: plain PSUM->SBUF cast on DVE offloads ACT
                        nc.vector.tensor_copy(ysbv, c3v)
                    else:
                        nc.scalar.activation(ysbv, c3v, AF.Identity,
                                             bias=b3_sb[:, m:m + 1], scale=1.0)
                    eng = nc.sync if m % 2 == 0 else nc.gpsimd
                    eng.dma_start(
                        y.ap()[:, NPX * m + HWPX * img:
                               NPX * m + HWPX * (img + 1)], ysb[:])

            def tap_view(taps, img, qq=None):
                # shifted padded-x view [128, L, rows, W]; tap step 1 (row
                # triplet) or PW (col triplet)
                t0 = taps[0]
                r0, c0 = divmod(t0, KS)
                step = 1 if (len(taps) == 1 or taps[1] == t0 + 1) else PW
                base = img * PIMG + r0 * PW + c0
                if qq is None:
                    rows = H
                else:
                    rows = 14
                    base += 14 * qq * PW
                return _sv(pad_t, pad_ps, base,
                           [(step, len(taps)), (PW, rows), (1, W)])

            # ---- involution, one image at a time ----
            for img in range(BL):
                o2 = pso.tile([128, 1024], f32, tag="o2", name=f"o2_{img}")

                filler_state = {"first": True}

                def pe_filler(n=1):
                    # dependency-free matmuls into the unused strip of the
                    # out2 psum bank; they keep the PE HAM clock-gate warm
                    # during DVE/ACT-bound stretches. First one per image
                    # starts the region (defined values).
                    if not FILLERS:
                        return
                    for _ in range(n):
                        nc.tensor.matmul(o2[:, 800:1024], id_sb,
                                         xsb[:, 0:224],
                                         start=filler_state["first"], stop=True,
                                         skip_group_check=True)
                        filler_state["first"] = False

                # accums commute: start flag = first emission per region,
                # stop pinned to tap 48 (always flushed last). gpsimd-made
                # products enter the queue 2 groups late so the PE never
                # waits on the slow Pool muls.
                fast_fifo, slow_fifo, started = [], [], {}

                def push_accum(prods, taps, gi, slow):
                    for s, t in enumerate(taps):
                        for (off, wd) in ((0, 512), (512, HWPX - 512)):
                            e = (prods, HWPX * s + off, t, off, wd)
                            if slow:
                                slow_fifo.append((gi + 2, e))
                            else:
                                fast_fifo.append(e)

                def _emit(e, stop=False):
                    pr, poff, t, off, wd = e
                    st = off not in started
                    started[off] = True
                    nc.tensor.matmul(
                        o2[:, off:off + wd], id_sb,
                        pr[:, poff:poff + wd],
                        start=st, stop=stop,
                        skip_group_check=True,
                    )

                def emit_accum(n, cur_gi):
                    while slow_fifo and slow_fifo[0][0] <= cur_gi:
                        fast_fifo.append(slow_fifo.pop(0)[1])
                    take = fast_fifo[:n]
                    del fast_fifo[:len(take)]
                    for e in take:
                        _emit(e)

                def flush_accum():
                    rest = fast_fifo + [e for _, e in slow_fifo]
                    fast_fifo.clear()
                    slow_fifo.clear()
                    for e in rest:
                        if e[2] != NTAP - 1:
                            _emit(e)
                    for e in rest:
                        if e[2] == NTAP - 1:
                            _emit(e, stop=True)

                for gi, taps in enumerate(GROUPS):
                    ns = len(taps)
                    direct = gi in direct_groups
                    on_gp = gi in gp_groups
                    prods = prodp.tile([128, ns * HWPX], bf16, tag="prod",
                                       name=f"prod{img}_{gi}")
                    wm = None
                    if not direct:
                        wm = wsmp.tile([128, ns * HWPX], bf16, tag="wm",
                                       name=f"wm{img}_{gi}")
                    for qq in range(2):
                        q = 2 * img + qq
                        pe_filler(1)
                        bq = psB.tile([128, 1536], f32, tag="bc",
                                      name=f"bc{img}_{gi}_{qq}")
                        for s, t in enumerate(taps):
                            nc.tensor.matmul(
                                bq[:, 512 * s:512 * s + QW],
                                wsd_sb[32 * s:32 * (s + 1),
                                       128 * gi:128 * (gi + 1)],
                                w1rep[32 * s:32 * (s + 1), QW * q:QW * (q + 1)],
                                start=True, stop=True,
                                tile_position=(32 * s, 0),
                            )
                        if gi >= 1:
                            emit_accum(ACCUM_PACE, gi)
                        if not direct:
                            # drain all strips to bf16 wm in one ACT op
                            bqv = _sv(bq, bq.ap[0][0], 0, [(512, ns), (1, QW)])
                            wmv = _sv(wm, wm.ap[0][0], QW * qq,
                                      [(HWPX, ns), (1, QW)])
                            nc.scalar.activation(wmv, bqv, AF.Identity,
                                                 bias=0.0, scale=1.0)
                        else:
                            # direct: DVE multiplies PSUM maps (fp32) in one op
                            bqv = _sv(bq, bq.ap[0][0], 0,
                                      [(512, ns), (W, 14), (1, W)])
                            prv = _sv(prods, prods.ap[0][0], QW * qq,
                                      [(HWPX, ns), (W, 14), (1, W)])
                            if use_stt:
                                for s, t in enumerate(taps):
                                    nc.vector.scalar_tensor_tensor(
                                        prods[:, HWPX * s + QW * qq:
                                              HWPX * s + QW * (qq + 1)]
                                        .rearrange("p (i j) -> p i j",
                                                   i=14, j=W),
                                        bq[:, 512 * s:512 * s + QW].rearrange(
                                            "p (i j) -> p i j", i=14, j=W),
                                        bsd_sb[:, t:t + 1],
                                        tap_view([t], img, qq),
                                        ALU.add, ALU.mult,
                                    )
                            else:
                                nc.vector.tensor_tensor(
                                    prv, bqv, tap_view(taps, img, qq),
                                    ALU.mult)
                    if not direct:
                        if on_gp:
                            # per-tap grain so accums unblock sooner
                            for s, t in enumerate(taps):
                                nc.gpsimd.tensor_tensor(
                                    prods[:, HWPX * s:HWPX * (s + 1)]
                                    .rearrange("p (i j) -> p i j", i=H, j=W),
                                    wm[:, HWPX * s:HWPX * (s + 1)]
                                    .rearrange("p (i j) -> p i j", i=H, j=W),
                                    tap_view([t], img), ALU.mult)
                        else:
                            wmv = _sv(wm, wm.ap[0][0], 0,
                                      [(HWPX, ns), (W, H), (1, W)])
                            prv = _sv(prods, prods.ap[0][0], 0,
                                      [(HWPX, ns), (W, H), (1, W)])
                            nc.vector.tensor_tensor(prv, wmv,
                                                    tap_view(taps, img),
                                                    ALU.mult)
                    push_accum(prods, taps, gi, on_gp)
                flush_accum()

                nc.scalar.activation(
                    out2sb[:, HWPX * img:HWPX * (img + 1)], o2[:, 0:HWPX],
                    AF.Identity, bias=0.0, scale=1.0)
                conv3_img(img)

    nc.compile()
    return nc


def get_program(all_direct=False):
    key = "nc_all_direct" if all_direct else "nc"
    if key not in _prog_cache:
        if all_direct:
            _prog_cache[key] = _build_program(
                frozenset(range(NGRP)), frozenset(), use_stt=True,
                fuse_y=False)
        else:
            _prog_cache[key] = _build_program()
    return _prog_cache[key]


def _host_prep(inputs):
    """Fold scales into weights; build per-core DRAM tensor layouts."""
    x = np.asarray(inputs["x"], np.float32)
    W1 = np.asarray(inputs["W1"], np.float32) * np.asarray(inputs["s1"], np.float32)[:, None]
    Wr = np.asarray(inputs["Wr"], np.float32) * np.asarray(inputs["sr"], np.float32)[:, None]
    Ws = np.asarray(inputs["Ws"], np.float32)
    W3 = np.asarray(inputs["W3"], np.float32) * np.asarray(inputs["s3"], np.float32)[:, None]
    b1 = np.asarray(inputs["b1"], np.float32)
    br = np.asarray(inputs["br"], np.float32)
    bs = np.asarray(inputs["bs"], np.float32)
    b3 = np.asarray(inputs["b3"], np.float32)

    w1t = np.ascontiguousarray(
        W1.T.reshape(4, 128, 128).transpose(1, 0, 2).reshape(128, 512)).astype(BF)
    wrt = np.tile(Wr.T, (1, 4)).astype(BF)
    wsd = np.zeros((96, NGRP * 128), np.float32)
    WsT = Ws.reshape(G, NTAP, CRED)  # [g, t, j]
    for gi, taps in enumerate(GROUPS):
        for s, t in enumerate(taps):
            blk = WsT[:, t, :].T  # [j, g]
            wsd[32 * s:32 * s + 32, 128 * gi:128 * (gi + 1)] = np.repeat(
                blk, GCH, axis=1)
    wsd = wsd.astype(BF)
    w3t = W3.T.astype(BF)
    ident = np.eye(128, dtype=np.float32).astype(BF)
    bsd = np.repeat(bs.reshape(G, NTAP), GCH, axis=0)
    bsd = np.ascontiguousarray(bsd).astype(np.float32)

    base = {
        "w1t": w1t, "wrt": wrt, "wsd": wsd, "w3t": w3t, "ident": ident,
        "b1": b1.reshape(128, 1).astype(np.float32),
        "brr": np.tile(br, 4).reshape(128, 1).astype(np.float32),
        "bsd": bsd,
        "b3": np.ascontiguousarray(b3.reshape(4, 128).T).astype(np.float32),
    }
    in_maps = []
    for c in range(N_CORES):
        xs = x[BL * c:BL * (c + 1)]
        # quarter-major: [128p, img, half, chunk, 392]
        arr = xs.reshape(BL, 4, 128, 2, 14 * W)
        xc = np.ascontiguousarray(
            arr.transpose(2, 0, 3, 1, 4).reshape(128, 4 * NPX)).astype(BF)
        m = dict(base)
        m["x"] = xc
        in_maps.append(m)
    return in_maps


def _unshard(results):
    out = np.empty((B, CIN, H, W), np.float32)
    for c in range(N_CORES):
        yc = results[c]["y"].astype(np.float32)
        yv = yc.reshape(128, 4, BL, H, W).transpose(2, 1, 0, 3, 4)
        out[BL * c:BL * (c + 1)] = yv.reshape(BL, CIN, H, W)
    return out


def kernel(**inputs):
    # the fast path assumes bs == 0 and b3 == 0 (true for this problem's
    # setup_inputs); otherwise route through the exact fallback program
    # (stt direct taps apply bs; ACT epilogue applies b3)
    all_direct = bool(np.abs(np.asarray(inputs["bs"])).max() > 0
                      or np.abs(np.asarray(inputs["b3"])).max() > 0)
    nc = get_program(all_direct)
    in_maps = _host_prep(inputs)
    import os
    trace = bool(os.environ.get("KERNEL_TRACE"))
    kw = {}
    if trace:
        import tempfile
        kw = dict(trace=True, tmpdir=tempfile.mkdtemp(prefix="ktr_"))
        try:
            import ntff_shim  # noqa: F401
        except ImportError:
            pass
    res = bass_utils.run_bass_kernel_spmd(
        nc, in_maps, core_ids=list(range(N_CORES)), **kw)
    if trace and res.exec_time_ns is not None:
        prof = os.environ.get("KERNEL_PROFILE_OUT")
        if prof:
            with open(prof, "w") as f:
                f.write(str(res.exec_time_ns))
        print(f"HW exec time: {res.exec_time_ns} ns")
    return _unshard(res.results)


# revision 27
# speedup vs baseline: 1.3130x; 1.3130x over previous
"""Trainium2 Bass kernel for nn_Block_36438502540029 (involution CNN block).

Structure per core (data-parallel over batch, 2 images/core):
  conv1 (1x1, 512->128) -> ReLU -> padded bf16 buffer
  reduce (1x1, 128->32, M-replicated x4) -> ReLU -> w1rep
  involution apply, processed per image (half-spatial) so the out2
    accumulator needs only 2 PSUM banks, leaving 3 ping-pong slots for
    the per-tap broadcast PSUM tiles:
      span+broadcast: per-tap stationary Ws_dup[32,128] (rows duplicated
        16x host-side), row-tiled pair matmuls -> kernel maps in PSUM
      drained pairs: ACT drains PSUM->SBUF bf16; DVE bf16 muls vs
        shifted padded x1
      direct pairs: DVE muls read the PSUM maps directly (fp32, 1x)
      PE identity-matmul accumulation into the half-image out2 PSUM
  conv3 (1x1, 128->512) + identity residual via extra matmul + bias

All matmuls bf16 (full rate); PSUM accumulation fp32. Dependency-free
filler matmuls into an unused PSUM strip keep the PE HAM clock-gate warm.
"""
import numpy as np
import ml_dtypes
from contextlib import ExitStack

import bass_rust
import concourse.bass as bass
import concourse.tile as tile
from concourse import bacc, mybir
from concourse import bass_utils

bf16 = mybir.dt.bfloat16
f32 = mybir.dt.float32
AF = mybir.ActivationFunctionType
ALU = mybir.AluOpType
BF = ml_dtypes.bfloat16

N_CORES = 8
B, CIN, H, W = 16, 512, 28, 28
BL = B // N_CORES            # images per core
CMID, CRED, G, GCH = 128, 32, 8, 16
KS, PD = 7, 3                # kernel size, pad
HWPX = H * W                 # 784
NPX = BL * HWPX              # 1568
PW = W + 2 * PD              # 34
PIMG = PW * PW               # 1156
NPAD = BL * PIMG + 8         # 2320 (slack for strided quarter views)
NTAP = KS * KS               # 49
NPAIR = (NTAP + 1) // 2      # 25 (last pair single)

CHUNKS = [(0, 512), (512, 512), (1024, 512), (1536, 32)]      # ragged 1568
HCHUNKS = [(0, 512), (512, 272)]                              # ragged 784
QW = 392                     # quarter width (half of one image)

# pairs whose taps use the direct-from-PSUM DVE path (no ACT drain)
DIRECT_PAIRS = frozenset({2, 5, 8, 11, 14, 17, 20, 23})

_prog_cache = {}


def _sv(ap_base, pstride, offset, dims):
    """Strided view with explicit (step, num) free dims."""
    v = ap_base[:, offset:offset + 1].copy()
    v.ap = bass_rust.VecI64Pair([[pstride, 128]] + [list(d) for d in dims])
    return v


def _build_program(direct_pairs=DIRECT_PAIRS, use_stt=False, fuse_y=True):
    nc = bacc.Bacc("TRN2", num_devices=N_CORES, debug=False)

    dr = {}
    dr["x"] = nc.dram_tensor("x", [128, 4 * NPX], bf16, kind="ExternalInput")
    dr["w1t"] = nc.dram_tensor("w1t", [128, 512], bf16, kind="ExternalInput")
    dr["wrt"] = nc.dram_tensor("wrt", [128, 128], bf16, kind="ExternalInput")
    dr["wsd"] = nc.dram_tensor("wsd", [64, NPAIR * 128], bf16, kind="ExternalInput")
    dr["w3t"] = nc.dram_tensor("w3t", [128, 512], bf16, kind="ExternalInput")
    dr["ident"] = nc.dram_tensor("ident", [128, 128], bf16, kind="ExternalInput")
    dr["b1"] = nc.dram_tensor("b1", [128, 1], f32, kind="ExternalInput")
    dr["brr"] = nc.dram_tensor("brr", [128, 1], f32, kind="ExternalInput")
    dr["bsd"] = nc.dram_tensor("bsd", [128, NTAP], f32, kind="ExternalInput")
    dr["b3"] = nc.dram_tensor("b3", [128, 4], f32, kind="ExternalInput")
    y = nc.dram_tensor("y", [128, 4 * NPX], bf16, kind="ExternalOutput")

    with tile.TileContext(nc) as tc:
        with ExitStack() as ctx:
            const = ctx.enter_context(tc.tile_pool(name="const", bufs=1))
            sbuf = ctx.enter_context(tc.tile_pool(name="sbuf", bufs=1))
            wsmp = ctx.enter_context(tc.tile_pool(name="wsm", bufs=6))
            prodp = ctx.enter_context(tc.tile_pool(name="prod", bufs=10))
            ystg = ctx.enter_context(tc.tile_pool(name="ystg", bufs=2))
            pso = ctx.enter_context(tc.tile_pool(name="pso", bufs=1, space="PSUM"))
            psB = ctx.enter_context(tc.tile_pool(name="psB", bufs=3, space="PSUM"))

            # ---- loads: x quarter 0 first so conv1 starts ASAP ----
            # x layout: [(img,half) quarter q: 4][chunk k: 4][392]
            xsb = sbuf.tile([128, 4 * NPX], bf16, name="xsb")
            nc.sync.dma_start(xsb[:, 0:NPX], dr["x"].ap()[:, 0:NPX])
            w1t_sb = const.tile([128, 512], bf16, name="w1t_sb")
            nc.scalar.dma_start(w1t_sb[:], dr["w1t"].ap())
            wrt_sb = const.tile([128, 128], bf16, name="wrt_sb")
            nc.gpsimd.dma_start(wrt_sb[:], dr["wrt"].ap())
            nc.scalar.dma_start(xsb[:, NPX:2 * NPX], dr["x"].ap()[:, NPX:2 * NPX])
            nc.gpsimd.dma_start(xsb[:, 2 * NPX:3 * NPX],
                                dr["x"].ap()[:, 2 * NPX:3 * NPX])
            nc.sync.dma_start(xsb[:, 3 * NPX:4 * NPX],
                              dr["x"].ap()[:, 3 * NPX:4 * NPX])
            wsd_sb = const.tile([128, NPAIR * 128], bf16, name="wsd_sb")
            nc.sync.dma_start(wsd_sb[0:64, :], dr["wsd"].ap())
            id_sb = const.tile([128, 128], bf16, name="id_sb")
            nc.sync.dma_start(id_sb[:], dr["ident"].ap())
            w3t_sb = const.tile([128, 512], bf16, name="w3t_sb")
            nc.gpsimd.dma_start(w3t_sb[:], dr["w3t"].ap())
            b1_sb = const.tile([128, 1], f32, name="b1_sb")
            nc.sync.dma_start(b1_sb[:], dr["b1"].ap())
            brr_sb = const.tile([128, 1], f32, name="brr_sb")
            nc.sync.dma_start(brr_sb[:], dr["brr"].ap())
            bsd_sb = const.tile([128, NTAP], f32, name="bsd_sb")
            nc.sync.dma_start(bsd_sb[:], dr["bsd"].ap())
            b3_sb = const.tile([128, 4], f32, name="b3_sb")
            nc.sync.dma_start(b3_sb[:], dr["b3"].ap())

            pad_t = sbuf.tile([128, NPAD], bf16, name="pad_t")
            nc.vector.memset(pad_t[:], 0.0)
            pad4 = pad_t[:, 0:BL * PIMG].rearrange(
                "p (b i j) -> p b i j", b=BL, i=PW, j=PW)

            # ---- conv1: out1 = relu(W1' @ x + b1); quarter-serial (392 px =
            # 14 rows, row-aligned for the strided pad write) through the
            # shared psum pool ----
            for q in range(4):
                b_, hh = q // 2, q % 2
                cps = psB.tile([128, 1024], f32, tag="bc", name=f"c1ps{q}")
                for k in range(4):
                    nc.tensor.matmul(
                        cps[:, 0:QW],
                        w1t_sb[:, 128 * k:128 * (k + 1)],
                        xsb[:, (4 * q + k) * QW:(4 * q + k + 1) * QW],
                        start=(k == 0), stop=(k == 3),
                    )
                nc.scalar.activation(
                    pad4[:, b_:b_ + 1, PD + 14 * hh:PD + 14 * hh + 14, PD:PD + W],
                    cps[:, 0:QW].rearrange("p (a i j) -> p a i j",
                                           a=1, i=14, j=W),
                    AF.Relu, bias=b1_sb[:], scale=1.0,
                )

            # ---- reduce: w1rep = relu(Wr'_rep @ out1 + br_rep), per quarter ----
            w1rep = sbuf.tile([128, NPX], bf16, name="w1rep")
            for q in range(4):
                b_, hh = q // 2, q % 2
                rps = psB.tile([128, 1024], f32, tag="bc", name=f"redps{q}")
                rhs = pad4[:, b_:b_ + 1, PD + 14 * hh:PD + 14 * hh + 14, PD:PD + W]
                nc.tensor.matmul(rps[:, 0:QW], wrt_sb, rhs,
                                 start=True, stop=True)
                nc.scalar.activation(
                    w1rep[:, QW * q:QW * (q + 1)], rps[:, 0:QW],
                    AF.Relu, bias=brr_sb[:], scale=1.0,
                )

            out2sb = sbuf.tile([128, NPX], bf16, name="out2sb")

            def conv3_half(hh):
                # y_m[img hh] = W3'_m @ out2 + x_m + b3_m; quarter strips at
                # 0/512 (x is quarter-major so the residual rhs is one slice)
                hoff = HWPX * hh
                for m in range(4):
                    c3 = psB.tile([128, 1024], f32, tag="bc", name=f"c3_{m}_{hh}")
                    for s2 in range(2):
                        q = 2 * hh + s2
                        nc.tensor.matmul(
                            c3[:, 512 * s2:512 * s2 + QW],
                            w3t_sb[:, 128 * m:128 * (m + 1)],
                            out2sb[:, hoff + QW * s2:hoff + QW * (s2 + 1)],
                            start=True, stop=False, skip_group_check=True,
                        )
                        nc.tensor.matmul(
                            c3[:, 512 * s2:512 * s2 + QW], id_sb,
                            xsb[:, (4 * q + m) * QW:(4 * q + m + 1) * QW],
                            start=False, stop=True, skip_group_check=True,
                        )
                    ysb = ystg.tile([128, HWPX], bf16, tag="y", name=f"y{m}_{hh}")
                    c3v = _sv(c3, c3.ap[0][0], 0, [(512, 2), (1, QW)])
                    ysbv = ysb[:].rearrange("p (s n) -> p s n", s=2, n=QW)
                    if fuse_y and m % 2 == 1:
                        # b3 == 0: plain PSUM->SBUF cast on DVE offloads ACT
                        nc.vector.tensor_copy(ysbv, c3v)
                    else:
                        nc.scalar.activation(ysbv, c3v, AF.Identity,
                                             bias=b3_sb[:, m:m + 1], scale=1.0)
                    eng = nc.sync if m % 2 == 0 else nc.gpsimd
                    eng.dma_start(y.ap()[:, NPX * m + hoff:NPX * m + hoff + HWPX],
                                  ysb[:])

            def pad_shift_half(t, himg, squeeze_q=None):
                di, dj = t // KS - PD, t % KS - PD
                if squeeze_q is None:
                    return pad4[:, himg:himg + 1,
                                PD + di:PD + di + H, PD + dj:PD + dj + W]
                hh = squeeze_q
                r0 = PD + di + 14 * hh
                off = himg * PIMG + r0 * PW + PD + dj
                return pad_t[:, off:off + 14 * PW].rearrange(
                    "p (i j) -> p i j", i=14, j=PW)[:, :, 0:W]

            # ---- involution apply, one image (half) at a time ----
            for himg in range(BL):
                o2 = pso.tile([128, 1024], f32, tag="o2", name=f"o2_{himg}")

                filler_state = {"first": True}

                def pe_filler(n=1):
                    # dependency-free matmuls into the unused strip of the
                    # out2 psum bank; they run whenever the PE would
                    # otherwise stall, keeping the HAM clock-gate warm.
                    # First one per half starts the region (defined values);
                    # it precedes the first real chunk-1 accum, whose
                    # start=True bank-clear wipes the junk harmlessly.
                    for _ in range(n):
                        nc.tensor.matmul(o2[:, 800:1024], id_sb,
                                         xsb[:, 0:224],
                                         start=filler_state["first"], stop=True,
                                         skip_group_check=True)
                        filler_state["first"] = False

                accum_fifo = []

                def push_accum(prods_taps):
                    for pr, t in prods_taps:
                        for ci in range(len(HCHUNKS)):
                            accum_fifo.append((pr, t, ci))

                def emit_accum(n):
                    take = accum_fifo[:n] if n else list(accum_fifo)
                    del accum_fifo[:len(take)]
                    for pr, t, ci in take:
                        off, wd = HCHUNKS[ci]
                        nc.tensor.matmul(
                            o2[:, off:off + wd], id_sb,
                            pr[:, off:off + wd],
                            start=(t == 0), stop=(t == NTAP - 1),
                            skip_group_check=True,
                        )

                for p in range(NPAIR):
                    taps = [t for t in (2 * p, 2 * p + 1) if t < NTAP]
                    ns = len(taps)
                    drained = p not in direct_pairs
                    prods = []
                    for s, t in enumerate(taps):
                        pr = prodp.tile([128, HWPX], bf16, tag="prod",
                                        name=f"prod{himg}_{t}")
                        prods.append(pr)
                    wm = None
                    if drained:
                        wm = wsmp.tile([128, ns * HWPX], bf16, tag="wm",
                                       name=f"wm{himg}_{p}")
                    for qq in range(2):
                        q = 2 * himg + qq
                        if qq == 0:
                            pe_filler(1)
                        bq = psB.tile([128, 1024], f32, tag="bc",
                                      name=f"bc{himg}_{p}_{qq}")
                        for s, t in enumerate(taps):
                            nc.tensor.matmul(
                                bq[:, 512 * s:512 * s + QW],
                                wsd_sb[32 * s:32 * (s + 1), 128 * p:128 * (p + 1)],
                                w1rep[32 * s:32 * (s + 1), QW * q:QW * (q + 1)],
                                start=True, stop=True,
                                tile_position=(32 * s, 0),
                            )
                        if p >= 2:
                            emit_accum(1)
                        if drained:
                            nc.scalar.activation(
                                wm[:].rearrange("p (s n) -> p s n",
                                                s=ns, n=HWPX)[
                                    :, :, QW * qq:QW * (qq + 1)],
                                bq[:].rearrange("p (s n) -> p s n",
                                                s=2, n=512)[:, 0:ns, 0:QW],
                                AF.Identity, bias=0.0, scale=1.0,
                            )
                        else:
                            for s, t in enumerate(taps):
                                if use_stt:
                                    nc.vector.scalar_tensor_tensor(
                                        prods[s][:, QW * qq:QW * (qq + 1)]
                                        .rearrange("p (i j) -> p i j",
                                                   i=14, j=W),
                                        bq[:, 512 * s:512 * s + QW].rearrange(
                                            "p (i j) -> p i j", i=14, j=W),
                                        bsd_sb[:, t:t + 1],
                                        pad_shift_half(t, himg, squeeze_q=qq),
                                        ALU.add, ALU.mult,
                                    )
                                else:
                                    nc.vector.tensor_mul(
                                        prods[s][:, QW * qq:QW * (qq + 1)]
                                        .rearrange("p (i j) -> p i j",
                                                   i=14, j=W),
                                        bq[:, 512 * s:512 * s + QW].rearrange(
                                            "p (i j) -> p i j", i=14, j=W),
                                        pad_shift_half(t, himg, squeeze_q=qq),
                                    )
                    if drained:
                        for s, t in enumerate(taps):
                            nc.vector.tensor_mul(
                                prods[s][:].rearrange("p (i j) -> p i j",
                                                      i=H, j=W),
                                wm[:, HWPX * s:HWPX * (s + 1)].rearrange(
                                    "p (i j) -> p i j", i=H, j=W),
                                pad_shift_half(t, himg),
                            )
                    push_accum([(prods[s], t) for s, t in enumerate(taps)])
                emit_accum(0)

                nc.scalar.activation(
                    out2sb[:, HWPX * himg:HWPX * (himg + 1)], o2[:, 0:HWPX],
                    AF.Identity, bias=0.0, scale=1.0)
                conv3_half(himg)


    nc.compile()
    return nc


def get_program(all_direct=False):
    key = "nc_all_direct" if all_direct else "nc"
    if key not in _prog_cache:
        if all_direct:
            _prog_cache[key] = _build_program(
                frozenset(range(NPAIR)), use_stt=True, fuse_y=False)
        else:
            _prog_cache[key] = _build_program()
    return _prog_cache[key]


def _host_prep(inputs):
    """Fold scales into weights; build per-core DRAM tensor layouts."""
    x = np.asarray(inputs["x"], np.float32)
    W1 = np.asarray(inputs["W1"], np.float32) * np.asarray(inputs["s1"], np.float32)[:, None]
    Wr = np.asarray(inputs["Wr"], np.float32) * np.asarray(inputs["sr"], np.float32)[:, None]
    Ws = np.asarray(inputs["Ws"], np.float32)
    W3 = np.asarray(inputs["W3"], np.float32) * np.asarray(inputs["s3"], np.float32)[:, None]
    b1 = np.asarray(inputs["b1"], np.float32)
    br = np.asarray(inputs["br"], np.float32)
    bs = np.asarray(inputs["bs"], np.float32)
    b3 = np.asarray(inputs["b3"], np.float32)

    w1t = np.ascontiguousarray(
        W1.T.reshape(4, 128, 128).transpose(1, 0, 2).reshape(128, 512)).astype(BF)
    wrt = np.tile(Wr.T, (1, 4)).astype(BF)
    wsd = np.zeros((64, NPAIR * 128), np.float32)
    WsT = Ws.reshape(G, NTAP, CRED)  # [g, t, j]
    for p in range(NPAIR):
        for s in range(2):
            t = 2 * p + s
            if t >= NTAP:
                continue
            blk = WsT[:, t, :].T  # [j, g]
            wsd[32 * s:32 * s + 32, 128 * p:128 * (p + 1)] = np.repeat(
                blk, GCH, axis=1)
    wsd = wsd.astype(BF)
    w3t = W3.T.astype(BF)
    ident = np.eye(128, dtype=np.float32).astype(BF)
    bsd = np.repeat(bs.reshape(G, NTAP), GCH, axis=0)
    bsd = np.ascontiguousarray(bsd).astype(np.float32)


    base = {
        "w1t": w1t, "wrt": wrt, "wsd": wsd, "w3t": w3t, "ident": ident,
        "b1": b1.reshape(128, 1).astype(np.float32),
        "brr": np.tile(br, 4).reshape(128, 1).astype(np.float32),
        "bsd": bsd,
        "b3": np.ascontiguousarray(b3.reshape(4, 128).T).astype(np.float32),
    }
    in_maps = []
    for c in range(N_CORES):
        xs = x[BL * c:BL * (c + 1)]
        # quarter-major: [128p, img, half, chunk, 392]
        arr = xs.reshape(BL, 4, 128, 2, 14 * W)
        xc = np.ascontiguousarray(
            arr.transpose(2, 0, 3, 1, 4).reshape(128, 4 * NPX)).astype(BF)
        m = dict(base)
        m["x"] = xc
        in_maps.append(m)
    return in_maps


def _unshard(results):
    out = np.empty((B, CIN, H, W), np.float32)
    for c in range(N_CORES):
        yc = results[c]["y"].astype(np.float32)
        yv = yc.reshape(128, 4, BL, H, W).transpose(2, 1, 0, 3, 4)
        out[BL * c:BL * (c + 1)] = yv.reshape(BL, CIN, H, W)
    return out


def kernel(**inputs):
    # the fast drained path assumes bs == 0 (true for this problem's
    # setup_inputs); nonzero bs routes every pair through the direct path,
    # which applies bs exactly
    all_direct = bool(np.abs(np.asarray(inputs["bs"])).max() > 0
                      or np.abs(np.asarray(inputs["b3"])).max() > 0)
    nc = get_program(all_direct)
    in_maps = _host_prep(inputs)
    import os
    trace = bool(os.environ.get("KERNEL_TRACE"))
    kw = {}
    if trace:
        import tempfile
        kw = dict(trace=True, tmpdir=tempfile.mkdtemp(prefix="ktr_"))
        try:
            import ntff_shim  # noqa: F401
        except ImportError:
            pass
    res = bass_utils.run_bass_kernel_spmd(
        nc, in_maps, core_ids=list(range(N_CORES)), **kw)
    if trace and res.exec_time_ns is not None:
        prof = os.environ.get("KERNEL_PROFILE_OUT")
        if prof:
            with open(prof, "w") as f:
                f.write(str(res.exec_time_ns))
        print(f"HW exec time: {res.exec_time_ns} ns")
    return _unshard(res.results)

